# revision 50
# baseline (speedup 1.0000x reference)
"""Trainium2 Bass kernel for nn_CPADConvOffsetStage.

The reference module is:
  up_posi = grid_sample_bilinear_border(posi_map -> [B,16,GP,GP], grid = base + offset*scale)
  h       = relu(w1 @ up_posi + b1)           (1x1 conv)
  weights = (w2 @ h + b2).reshape(B,64,9,H,W) (1x1 conv -> per-pixel 3x3 kernels)
  x_adapt = w_ca @ x                          (1x1 conv)
  out     = sum_k weights[:,:,k] * unfold3x3(x_adapt)[:,:,k] + bias

In setup_inputs() posi_map is spatially constant per channel (jnp.ones).
Bilinear interpolation weights sum to exactly 1, so up_posi is spatially
constant => h, weights are spatially constant => the whole module reduces
to ONE dense 3x3 convolution with host-precomputable weights
    Wfull[o,c,k] = wk[o,k] * w_ca[o,c],   wk = (w2 @ relu(w1 @ v + b1) + b2)
plus the bias.  The kernel below runs that conv data-parallel over batch
(1 batch image per NeuronCore, 8 cores).

If posi_map is NOT per-channel spatially constant (never the case for the
shipped setup_inputs), we fall back to an exact numpy port of the
reference.
"""

import os
import numpy as np
from contextlib import ExitStack

import concourse.bass as bass
import concourse.tile as tile
from concourse import mybir
from concourse.bass_utils import run_bass_kernel_spmd

# Problem constants (hardcoded per contract)
B, C, H, W = 8, 64, 128, 128
OC = 64
KK = 3
POSI_CH, GP = 16, 16
NCORES = 8
F32 = mybir.dt.float32

HPAD, WPAD = H + 2, W + 2      # host-padded image (130 x 130)
ROWS_PER_TILE = 4              # 4 rows * 128 cols = 512 = max fp32 moving free dim
NFREE = ROWS_PER_TILE * W      # 512
RB = 16                        # output rows per SBUF block
NBLK = H // RB                 # blocks per image
SUB = RB // ROWS_PER_TILE      # psum tiles per block
NXB = 4                        # rotated input block buffers
F32R = mybir.dt.float32r       # fp32 storage, single-pass relaxed-precision matmul

_cached_nc = None
_cached_variant = None
last_results = None            # test harness introspection


def _ensure_ntff_hook():
    """Register the axon NTFF-profile hook that this image's antenv lacks.

    run_bass_kernel_spmd(trace=True) under axon needs
    antenv.axon_hooks.get_axon_ntff_profile_hook; the hook machinery
    exists in trn_agent_boot but was never registered because
    antenv.axon_hooks is missing.  Recreate the module in sys.modules.
    """
    import sys
    import types

    if "antenv.axon_hooks" in sys.modules:
        return
    try:
        from trn_agent_boot.trn_boot import _ntff_profile_via_ctypes

        hook = _ntff_profile_via_ctypes("/opt/axon/libaxon_pjrt.so")
    except Exception:
        hook = None
    mod = types.ModuleType("antenv.axon_hooks")
    mod.get_axon_ntff_profile_hook = lambda: hook
    mod.set_axon_ntff_profile_hook = lambda h: None
    sys.modules["antenv.axon_hooks"] = mod
    try:
        import antenv

        antenv.axon_hooks = mod
    except Exception:
        pass


def _build_conv_nc(variant="bf16_tiled", split_waits=True):
    """3x3 conv, 64->64 ch, on one (host-padded) [64,130,130] image.

    SPMD over 8 cores, one batch image per core.  The input arrives
    zero-padded from the host so every 3x3 tap is a plain shifted
    window read; no memsets / halo special cases on device (this also
    keeps every Matmult at <=2 semaphore waits — walrus rejects 3+ on
    the LDWEIGHTS struct).
    """
    o16 = variant.endswith("_o16")
    base_variant = variant[:-4] if o16 else variant
    tiled = base_variant in ("bf16_tiled", "bf16_dup3")
    dup = base_variant == "bf16_dup3"
    mm_dt = mybir.dt.bfloat16 if tiled else F32R
    o_dt = mybir.dt.bfloat16 if o16 else F32
    xb_parts = 128 if dup else C
    w_cols = 6 * OC if dup else 9 * OC
    nc = bass.Bass()
    x_d = nc.declare_dram_parameter("x", [C, HPAD, WPAD], mm_dt, isOutput=False)
    w_d = nc.declare_dram_parameter("wts", [xb_parts, w_cols], mm_dt, isOutput=False)
    b_d = nc.declare_dram_parameter("wb", [OC, 1], F32, isOutput=False)
    o_d = nc.declare_dram_parameter("out", [OC, H, W], o_dt, isOutput=True)

    with ExitStack() as ctx:
        tc = ctx.enter_context(tile.TileContext(nc))
        singles = ctx.enter_context(tc.tile_pool(name="singles", bufs=1))
        outs = ctx.enter_context(tc.tile_pool(name="outs", bufs=4))
        psum = ctx.enter_context(
            tc.tile_pool(name="psum", bufs=8 if dup else 4, space="PSUM")
        )
        tmps = ctx.enter_context(tc.tile_pool(name="tmps", bufs=4))

        w_sb = singles.tile([xb_parts, w_cols], mm_dt)
        nc.sync.dma_start(out=w_sb[:, :], in_=w_d[:, :])
        b_sb = singles.tile([OC, 1], F32)
        nc.sync.dma_start(out=b_sb[:, :], in_=b_d[:, :])

        # Rotated input blocks: 18 padded rows each (16 output rows + halo)
        xbs = []
        for i in range(NXB):
            xb_buf = singles.tile([xb_parts, RB + 2, WPAD], mm_dt, tag=f"xb{i}", name=f"xb{i}")
            if dup:
                # The duplicate (shifted) half never receives its last
                # column from DMA; zero it once so K=128 solo matmuls
                # (whose bottom-half weights are zero) can't hit NaN*0.
                nc.vector.memset(xb_buf[C : 2 * C, :, WPAD - 1 : WPAD], 0.0)
            xbs.append(xb_buf)

        # Tap k -> column group: A (psum partitions 0:64, tile_position (0,0))
        # gets taps 0,2,4,6,8; B (64:128, (0,64)) gets 1,3,5,7.  The two
        # column groups of the 128x128 PE array run concurrently (separate
        # XBUS streams), so 9 taps cost ~5 matmul slots instead of 9.
        for blk in range(NBLK):
            xb = xbs[blk % NXB]
            # Prefetch the input pipeline aggressively: schedule each block's
            # load + shifted-duplicate copy as if issued a block earlier, so
            # the first matmul of a block never waits on them (the profile
            # showed ~2.5us PE gaps at early block boundaries).
            with tc.high_priority(offset=60):
                nc.sync.dma_start(
                    out=xb[0:C, :, :],
                    in_=x_d[:, blk * RB : blk * RB + RB + 2, :],
                )
                if dup:
                    # partitions 64:128 = same rows shifted one column left,
                    # so a K=128 matmul contracts two horizontally-adjacent
                    # taps at once.  Derived on-chip (cross-partition copy)
                    # instead of a second DRAM read — DMA is the bottleneck.
                    nc.vector.tensor_copy(
                        xb[C : 2 * C, :, 0 : WPAD - 1],
                        xb[0:C, :, 1:WPAD],
                    )
            o_blk = outs.tile([OC, RB * W], o_dt)
            for s in range(SUB):
                r0 = s * ROWS_PER_TILE        # row offset within block
                if dup:
                    ps = psum.tile([128, NFREE], F32)
                    # 6 all-K=128 matmuls -> 3 col-tiled slots (K=128 streams
                    # ~2x faster per column than K=64 on this part, and the
                    # solo taps' bottom-half weights are zero):
                    #  A: pair(0,1) | pair(6,7) | solo k5
                    #  B: pair(3,4) | solo k2   | solo k8
                    mms = [
                        (0, 0, 0, True, False),   # pair row 0 -> A
                        (1, 1, 1, True, False),   # pair row 1 -> B
                        (2, 2, 0, False, False),  # pair row 2 -> A
                        (3, 0, 1, False, False),  # k2 (row 0, col 2) -> B
                        (4, 1, 0, False, True),   # k5 (row 1, col 2) -> A
                        (5, 2, 1, False, True),   # k8 (row 2, col 2) -> B
                    ]
                    for wi, i, col, st, sp in mms:
                        j = 0 if wi < 3 else 2
                        rhs = xb[:, r0 + i : r0 + i + ROWS_PER_TILE, j : j + W]
                        nc.tensor.matmul(
                            ps[64 * col : 64 * col + OC, :],
                            lhsT=w_sb[:, wi * OC : (wi + 1) * OC],
                            rhs=rhs,
                            start=st,
                            stop=sp,
                            tile_position=(0, 64 * col),
                            skip_group_check=True,
                        )
                elif tiled:
                    ps = psum.tile([128, NFREE], F32)
                    for k in range(9):
                        i, j = divmod(k, 3)
                        rhs = xb[:, r0 + i : r0 + i + ROWS_PER_TILE, j : j + W]
                        col = k % 2           # even taps -> A, odd -> B
                        nc.tensor.matmul(
                            ps[64 * col : 64 * col + OC, :],
                            lhsT=w_sb[:, k * OC : (k + 1) * OC],
                            rhs=rhs,
                            start=(k < 2),
                            stop=(k >= 7),
                            tile_position=(0, 64 * col),
                            skip_group_check=True,
                        )
                if tiled:
                    # Fold: B half evicted by ACT (with bias), A half added
                    # by DVE (walrus allows only one PSUM input per DVE op).
                    ps_a = ps[0:OC, 0:NFREE]
                    ps_b = ps[64:128, 0:NFREE]
                    tmpb = tmps.tile([OC, NFREE], F32)
                    nc.scalar.activation(
                        out=tmpb[:, :],
                        in_=ps_b,
                        func=mybir.ActivationFunctionType.Identity,
                        bias=b_sb[:, 0:1],
                        scale=1.0,
                    )
                    nc.vector.tensor_add(
                        o_blk[:, s * NFREE : (s + 1) * NFREE],
                        ps_a,
                        tmpb[:, :],
                    )
                else:
                    ps = psum.tile([OC, NFREE], F32)
                    for k in range(9):
                        i, j = divmod(k, 3)
                        rhs = xb[:, r0 + i : r0 + i + ROWS_PER_TILE, j : j + W]
                        nc.tensor.matmul(
                            ps[:, :],
                            lhsT=w_sb[:, k * OC : (k + 1) * OC],
                            rhs=rhs,
                            start=(k == 0),
                            stop=(k == 8),
                        )
                    nc.scalar.activation(
                        out=o_blk[:, s * NFREE : (s + 1) * NFREE],
                        in_=ps[:, :],
                        func=mybir.ActivationFunctionType.Identity,
                        bias=b_sb[:, 0:1],
                        scale=1.0,
                    )
            # Two half-block output DMAs: the first half drains while the
            # second half is still being computed (shorter kernel tail).
            HRB = RB // 2
            for h in range(2):
                nc.sync.dma_start(
                    out=o_d[:, blk * RB + h * HRB : blk * RB + (h + 1) * HRB, :],
                    in_=o_blk[:, h * HRB * W : (h + 1) * HRB * W].rearrange(
                        "p (r w) -> p r w", r=HRB
                    ),
                )
    if split_waits:
        _split_sync_waits(nc)
    return nc


def _split_sync_waits(nc, limit=1):
    """Hoist extra sync waits onto injected wait-only EventSemaphore ops.

    The neuronxcc walrus used under axon rejects compute instructions
    carrying more than one sync wait ("Too many sync wait commands", e.g.
    S3_LW / S3D3_AC structs).  Tile's sem assignment emits up to ~3.
    For every instruction with >limit waits, keep the first `limit` and
    prepend one wait-only EventSemaphore per extra wait on the same
    engine (same program position => same semantics).
    """
    import copy as _copy

    f = nc.m.functions[0]
    template = None
    for blk in f.blocks:
        for inst in blk.instructions:
            if type(inst).__name__ == "InstEventSemaphore":
                template = inst
                break
        if template is not None:
            break
    if template is None:
        return
    n_split = 0
    for blk in f.blocks:
        new_list = []
        changed = False
        for inst in blk.instructions:
            si = getattr(inst, "sync_info", None)
            op = str(getattr(inst, "opcode", ""))
            waits = list(si.on_wait) if (si and si.on_wait) else []
            if len(waits) > limit:
                for w in waits[limit:]:
                    ev = _copy.deepcopy(template)
                    ev.name = f"waitsplit_{n_split}"
                    n_split += 1
                    ev.engine = inst.engine
                    ev.sync_info = mybir.SyncInfo(on_wait=[w], on_update=[])
                    new_list.append(ev)
                inst.sync_info = mybir.SyncInfo(
                    on_wait=waits[:limit], on_update=list(si.on_update or [])
                )
                changed = True
            new_list.append(inst)
        if changed:
            blk.instructions = new_list


def _dma_out_engine(nc):
    """Engine whose queue carries output-DMA triggers (env-selectable)."""
    name = os.environ.get("BASS_OUT_DMA_ENGINE", "gpsimd")
    return getattr(nc, name)


def _build_spatial_nc(nwarm=12):
    """3x3 conv via spatially-split PE column groups (no PSUM fold).
    Measured 37.2us on trn2 (vs 49.7us for the fold-based baseline).

    One [64,130,130] host-padded bf16 image per core.  The two 64-wide
    PE column groups compute DIFFERENT output rows (A: first half of a
    round, B: second half), each accumulating all 9 taps of its tile
    into its own PSUM half — the baseline's expensive ACT+DVE fold of
    partial sums disappears; a single ACT Identity(+bias) evicts both
    halves at once.

    Taps: 3 horizontal pairs as K=128 matmuls (partitions 64:128 hold a
    one-column-left-shifted on-chip duplicate of the image), and the 3
    remaining taps as true K=64 matmuls row-tiled to PE row positions
    0/64 (the shifted duplicate can serve column-2 taps at row position
    64), so 4 K=64 quadrant matmuls stream concurrently.  Streamed
    columns per 4-row tile: 3*512 (pairs) + 1.5*512 (solos) = 2304
    vs 3072 for the zero-padded-solo variant.

    Rounds over output rows: 16 + 32 + 32 + 32 + 16 (small first round
    so real matmuls start early; small last round for a short tail).
    ~18 warmup matmuls on a zeroed scratch tile keep the PE HAM
    un-throttled while the first input block loads.  Output is bf16.
    """
    solo_mode = os.environ.get("BASS_SOLO_MODE", "rt")
    x_fp8 = os.environ.get("BASS_X_FP8", "0") == "1"
    nc = bass.Bass()
    BF16 = mybir.dt.bfloat16
    XDT = mybir.dt.float8e4 if x_fp8 else BF16
    x_d = nc.declare_dram_parameter("x", [C, HPAD, WPAD], XDT, isOutput=False)
    # one combined constant tensor (one DMA trigger): pair weights 0:192,
    # solo weights 192:384, per-channel bias col 384
    w_d = nc.declare_dram_parameter("w", [128, 6 * OC + 1], BF16, isOutput=False)
    o_d = nc.declare_dram_parameter("out", [OC, H, W], BF16, isOutput=True)

    # rounds: (start_row, n_rows) — two small rounds first so early
    # compute starts on little data while the input pipeline ramps
    rounds = [(0, 16), (16, 16), (32, 32), (64, 32), (96, 32)]

    with ExitStack() as ctx:
        tc = ctx.enter_context(tile.TileContext(nc))
        singles = ctx.enter_context(tc.tile_pool(name="singles", bufs=1))
        xbig = ctx.enter_context(tc.tile_pool(name="xbig", bufs=3))
        xsmall = ctx.enter_context(tc.tile_pool(name="xsmall", bufs=2))
        obig = ctx.enter_context(tc.tile_pool(name="obig", bufs=3))
        osmall = ctx.enter_context(tc.tile_pool(name="osmall", bufs=2))
        psum = ctx.enter_context(tc.tile_pool(name="psum", bufs=7, space="PSUM"))
        wpsum = ctx.enter_context(tc.tile_pool(name="wpsum", bufs=1, space="PSUM"))

        w_sb = singles.tile([128, 6 * OC + 1], BF16)
        scratch = singles.tile([128, 512], BF16)
        with tc.high_priority():
            nc.gpsimd.memset(scratch[:, :], 0.0)
        with tc.high_priority(offset=2000):
            # weights must beat the (also-hoisted) input blocks into the
            # DMA queues — everything downstream waits on them
            nc.sync.dma_start(out=w_sb[:, :], in_=w_d[:, :])
        wp_sb = w_sb[:, 0 : 3 * OC]
        ws_sb = w_sb[:, 3 * OC : 6 * OC]
        b_sb = w_sb[:, 6 * OC : 6 * OC + 1]
        b32_sb = singles.tile([128, 1], F32)
        nc.gpsimd.tensor_copy(b32_sb[:, :], b_sb)   # DVE wants fp32 bias

        # PE warmup: un-throttle the HAM clock gate while input DMA runs
        # (the warmup PSUM bank is never read).
        wps = wpsum.tile([128, 512], F32)
        for i in range(nwarm):
            g = 64 * (i % 2)
            nc.tensor.matmul(
                wps[g : g + OC, :],
                lhsT=scratch[:, 0:OC],
                rhs=scratch[:, :],
                start=True,
                stop=True,
                tile_position=(0, g),
                skip_group_check=True,
            )

        for R, NR in rounds:
            nrp = NR + 2                       # padded input rows incl. halo
            half = NR // 2                     # rows per column group
            nchunk = NR // 16                  # 2 tiles/group per chunk
            xpool = xbig if NR == 32 else xsmall
            opool = obig if NR == 32 else osmall
            xb = xpool.tile([128, nrp, WPAD], XDT)
            # round 0: split the load in two so the A-lane (first `half`
            # rows + halo) can start while the B half is still in flight,
            # and outrank even the weights in the DMA queues
            splits = (
                [(0, half + 2), (half + 2, nrp)] if R == 0 else [(0, nrp)]
            )
            with tc.high_priority(offset=3000 if R == 0 else 60):
                for r0_, r1_ in splits:
                    nc.sync.dma_start(
                        out=xb[0:C, r0_:r1_, :],
                        in_=x_d[:, R + r0_ : R + r1_, :],
                    )
                    # partitions 64:128 = one column left-shifted duplicate
                    # so a K=128 matmul contracts two adjacent taps.
                    nc.vector.tensor_copy(
                        xb[C : 2 * C, r0_:r1_, 0 : WPAD - 1],
                        xb[0:C, r0_:r1_, 1:WPAD],
                    )
                if solo_mode == "pad":
                    # zero-padded K=128 solos read dup col WPAD-1 (weights
                    # are zero there but NaN*0 would poison the sum)
                    nc.vector.memset(xb[C : 2 * C, :, WPAD - 1 : WPAD], 0.0)
            o_blk = opool.tile([128, half * W], BF16)

            ntile = 2 * nchunk                 # tiles per column group
            # PSUM bank t: partitions 0:64 = group-A tile t (round rows
            # 4t..4t+4), partitions 64:128 = group-B tile t (+half rows).
            pss = [
                psum.tile([128, NFREE], F32, tag="ps", name=f"ps{R}_{k}")
                for k in range(ntile)
            ]

            def pair_mm(j, t, g):
                r0 = 4 * t + (half if g else 0)
                return (
                    xb[0 : 2 * C, r0 + j : r0 + j + 4, 0:W],
                    wp_sb[:, j * OC : (j + 1) * OC],
                    (0, g),
                )

            def solo_mm(i, t, g, rp):
                """Tap 3i+2 as K=64 at PE row position rp (0 or 64);
                the shifted duplicate serves it at rp=64 (column 1)."""
                r0 = 4 * t + (half if g else 0)
                if solo_mode == "pad":
                    return (
                        xb[0 : 2 * C, r0 + i : r0 + i + 4, 2:WPAD],
                        ws_sb[:, i * OC : (i + 1) * OC],
                        (0, g),
                    )
                if rp == 0:
                    rhs = xb[0:C, r0 + i : r0 + i + 4, 2:WPAD]
                else:
                    rhs = xb[C : 2 * C, r0 + i : r0 + i + 4, 1 : WPAD - 1]
                return (
                    rhs,
                    ws_sb[rp : rp + C, i * OC : (i + 1) * OC],
                    (rp, g),
                )

            # All solos of the round first, then all pairs: the PE drains
            # on every 128-row <-> 64-row-tiled mode switch (~150ns), so
            # switch once per round, not once per chunk.  Solos alternate
            # tile parity = alternating row position AND psum bank — row
            # tiles may run concurrently only on different banks.
            rpb = 0 if solo_mode == "rt0" else 64
            seqs = []
            for g in (0, 64):
                lst = []
                for t in range(ntile):
                    rp = rpb if (t % 2) else 0
                    for i in range(3):
                        lst.append((t, solo_mm(i, t, g, rp), i == 0, False))
                for t in range(ntile):
                    for j in range(3):
                        lst.append((t, pair_mm(j, t, g), False, j == 2))
                # reorder solos: interleave tile pairs (t even, t odd) so
                # consecutive solos hit different row positions and banks
                so = []
                for ck in range(nchunk):
                    a = lst[6 * ck : 6 * ck + 3]
                    bl = lst[6 * ck + 3 : 6 * ck + 6]
                    for x, y in zip(a, bl):
                        so.extend((x, y))
                seqs.append(so + lst[6 * nchunk :])
            for mmA, mmB in zip(*seqs):
                for g, (t, m, st, sp) in ((0, mmA), (64, mmB)):
                    rhs, lhsT, tpos = m
                    nc.tensor.matmul(
                        pss[t][g : g + OC, :],
                        lhsT=lhsT,
                        rhs=rhs,
                        start=st,
                        stop=sp,
                        tile_position=(tpos[0], g),
                        skip_group_check=True,
                    )
            # evict each bank (both column-group halves + bias in one op),
            # alternating Scalar/Vector so neither engine gates the tail
            for k in range(ntile):
                dst = o_blk[:, k * NFREE : (k + 1) * NFREE]
                if k % 2 == 0:
                    nc.scalar.activation(
                        out=dst,
                        in_=pss[k][:, :],
                        func=mybir.ActivationFunctionType.Identity,
                        bias=b_sb[:, 0:1],
                        scale=1.0,
                    )
                else:
                    nc.vector.tensor_scalar_add(
                        dst, pss[k][:, :], b32_sb[:, 0:1]
                    )
            # per-chunk output DMA (starts while later chunks still compute);
            # triggers alternate engines so descriptor generation overlaps.
            # The FINAL round's triggers all go to Scalar: GpSimd has a
            # ~2.8us teardown drain that starts only after its last
            # instruction, so keeping it off the last round lets that drain
            # hide under compute.  (Scalar is safe here — unlike Sync it
            # carries no input-DMA triggers that eviction waits could block.)
            for ck in range(nchunk):
                rows = 8 if nchunk > 1 else half
                for g, roff in ((0, 0), (64, half)):
                    if R == rounds[-1][0]:
                        eng = nc.scalar
                    else:
                        eng = (
                            nc.scalar
                            if (ck + (g > 0)) % 2
                            else _dma_out_engine(nc)
                        )
                    eng.dma_start(
                        out=o_d[
                            :,
                            R + roff + rows * ck : R + roff + rows * (ck + 1),
                            :,
                        ],
                        in_=o_blk[
                            g : g + OC, ck * 2 * NFREE : (ck + 1) * 2 * NFREE
                        ].rearrange("p (r w) -> p r w", r=rows),
                    )
    _split_sync_waits(nc)
    return nc


def _pack_spatial(wts, solo_mode="rt"):
    """Split tap-major [C, 9*OC] lhsT into pair/solo stationary layouts.

    w_pairs [128, 3*OC]: K=128 blocks, taps (3j, 3j+1) stacked on the
    partition axis (base image / shifted duplicate).
    w_solo  [128, 3*OC]: solo taps (3i+2); duplicated on BOTH partition
    halves for rt mode (same weights serve PE row positions 0 and 64),
    zero bottom half for pad mode (K=128 matmuls, dup half inert).
    """
    wpair = np.zeros((128, 3 * OC), np.float32)
    wsolo = np.zeros((128, 3 * OC), np.float32)
    for j in range(3):
        wpair[0:C, j * OC:(j + 1) * OC] = wts[:, (3 * j) * OC:(3 * j + 1) * OC]
        wpair[C:2 * C, j * OC:(j + 1) * OC] = wts[:, (3 * j + 1) * OC:(3 * j + 2) * OC]
        solo = wts[:, (3 * j + 2) * OC:(3 * j + 3) * OC]
        wsolo[0:C, j * OC:(j + 1) * OC] = solo
        if solo_mode != "pad":
            wsolo[C:2 * C, j * OC:(j + 1) * OC] = solo
    return wpair, wsolo


def _host_conv_weights(posi_map, w1, b1, w2, b2, w_ca, bias):
    """Collapse the constant-posi_map weight generator on the host."""
    pm = np.asarray(posi_map, np.float64)[0]              # [16, GP, GP]
    vvec = pm.reshape(POSI_CH, -1)[:, 0]                  # per-channel constant
    h = np.maximum(np.asarray(w1, np.float64) @ vvec + np.asarray(b1, np.float64), 0.0)
    wvec = np.asarray(w2, np.float64) @ h + np.asarray(b2, np.float64)   # [576]
    wk = wvec.reshape(OC, 9)                              # [o, k]
    wca = np.asarray(w_ca, np.float64)                    # [o, c]
    wfull = wk[:, None, :] * wca[:, :, None]              # [o, c, k]
    wts = np.ascontiguousarray(
        wfull.transpose(1, 2, 0).reshape(C, 9 * OC).astype(np.float32)
    )                                                     # [c, k*OC + o]
    wb = np.ascontiguousarray(
        np.asarray(bias, np.float32).reshape(OC, 1)
    )
    return wts, wb


def _pack_dup3(wts):
    """Repack [C, 9*OC] tap-major lhsT into the dup3 layout [128, 6*OC].

    Columns 0:3*OC are K=128 pairs (taps (3p, 3p+1) stacked on the
    partition axis, matching the +1-column-shifted input duplicate);
    columns 3*OC:6*OC are the K=64 solo taps (3q+2), bottom half zero.
    """
    w3 = np.zeros((128, 6 * OC), np.float32)
    for p in range(3):
        w3[0:C, p * OC:(p + 1) * OC] = wts[:, (3 * p) * OC:(3 * p + 1) * OC]
        w3[C:2 * C, p * OC:(p + 1) * OC] = wts[:, (3 * p + 1) * OC:(3 * p + 2) * OC]
        w3[0:C, (3 + p) * OC:(4 + p) * OC] = wts[:, (3 * p + 2) * OC:(3 * p + 3) * OC]
    return w3


def _numpy_reference(x, offset, posi_map, w1, b1, w2, b2, w_ca, bias):
    """Exact numpy port of reference.py (general-input fallback)."""
    x = np.asarray(x, np.float32)
    offset = np.asarray(offset, np.float32)
    posi_map = np.asarray(posi_map, np.float32)
    w1 = np.asarray(w1, np.float32)
    b1 = np.asarray(b1, np.float32)
    w2 = np.asarray(w2, np.float32)
    b2 = np.asarray(b2, np.float32)
    w_ca = np.asarray(w_ca, np.float32)
    bias = np.asarray(bias, np.float32)

    Bq, _, Hq, Wq = x.shape
    dx = offset[:, 0] * (2.0 / max(Wq - 1, 1)) * 0.5
    dy = offset[:, 1] * (2.0 / max(Hq - 1, 1)) * 0.5
    ys = np.linspace(-1.0, 1.0, Hq, dtype=x.dtype)
    xs = np.linspace(-1.0, 1.0, Wq, dtype=x.dtype)
    gx = xs[None, None, :] + dx
    gy = ys[None, :, None] + dy
    img = np.broadcast_to(posi_map, (Bq, posi_map.shape[1], GP, GP))

    Hp = Wp = GP
    imgT = img.transpose(0, 2, 3, 1)                      # [B, Hp, Wp, C]
    ix = np.clip((gx + 1.0) * 0.5 * (Wp - 1), 0.0, Wp - 1)
    iy = np.clip((gy + 1.0) * 0.5 * (Hp - 1), 0.0, Hp - 1)
    x0 = np.floor(ix).astype(np.int32)
    y0 = np.floor(iy).astype(np.int32)
    x1 = np.minimum(x0 + 1, Wp - 1)
    y1 = np.minimum(y0 + 1, Hp - 1)
    wx = (ix - x0.astype(ix.dtype))[..., None]
    wy = (iy - y0.astype(iy.dtype))[..., None]
    bb = np.arange(Bq)[:, None, None]
    v00 = imgT[bb, y0, x0]
    v01 = imgT[bb, y0, x1]
    v10 = imgT[bb, y1, x0]
    v11 = imgT[bb, y1, x1]
    top = v00 * (1 - wx) + v01 * wx
    bot = v10 * (1 - wx) + v11 * wx
    up = (top * (1 - wy) + bot * wy).transpose(0, 3, 1, 2)  # [B, 16, H, W]

    h = np.maximum(np.einsum('oc,bchw->bohw', w1, up) + b1[None, :, None, None], 0.0)
    weights = np.einsum('oc,bchw->bohw', w2, h) + b2[None, :, None, None]
    weights = weights.reshape(Bq, OC, KK * KK, Hq, Wq)
    x_adapt = np.einsum('oc,bchw->bohw', w_ca, x)
    xp = np.pad(x_adapt, ((0, 0), (0, 0), (1, 1), (1, 1)))
    patches = np.stack(
        [xp[:, :, i:i + Hq, j:j + Wq] for i in range(KK) for j in range(KK)],
        axis=2,
    )
    out = (weights * patches).sum(axis=2) + bias
    return out.astype(np.float32)


def kernel(**inputs):
    global _cached_nc, last_results
    x = np.ascontiguousarray(np.asarray(inputs["x"], np.float32))
    posi_map = np.asarray(inputs["posi_map"], np.float32)

    per_ch = posi_map.reshape(posi_map.shape[0] * posi_map.shape[1], -1)
    if not np.all(per_ch == per_ch[:, :1]):
        # general (spatially varying posi_map) fallback: exact numpy port
        return _numpy_reference(**{k: inputs[k] for k in (
            "x", "offset", "posi_map", "w1", "b1", "w2", "b2", "w_ca", "bias")})

    wts, wb = _host_conv_weights(
        posi_map, inputs["w1"], inputs["b1"], inputs["w2"], inputs["b2"],
        inputs["w_ca"], inputs["bias"],
    )

    variant = os.environ.get("BASS_KERNEL_VARIANT", "spatial_rt")
    global _cached_variant
    if _cached_nc is None or _cached_variant != variant:
        if variant == "spatial_rt":
            _cached_nc = _build_spatial_nc(
                nwarm=int(os.environ.get("BASS_NWARM", "12"))
            )
        else:
            _cached_nc = _build_conv_nc(variant)
        _cached_variant = variant

    xpad = np.pad(x, ((0, 0), (0, 0), (1, 1), (1, 1)))
    import ml_dtypes

    if variant == "spatial_rt":
        wpair, wsolo = _pack_spatial(
            wts, os.environ.get("BASS_SOLO_MODE", "rt")
        )
        if os.environ.get("BASS_X_FP8", "0") == "1":
            xpad = xpad.astype(ml_dtypes.float8_e4m3fn)
        else:
            xpad = xpad.astype(ml_dtypes.bfloat16)
        wall = np.concatenate(
            [wpair, wsolo, np.concatenate([wb, wb], axis=0)], axis=1
        ).astype(ml_dtypes.bfloat16)
        in_maps = [{"x": xpad[i], "w": wall} for i in range(NCORES)]
    else:
        base_variant = variant[:-4] if variant.endswith("_o16") else variant
        if base_variant == "bf16_dup3":
            wts = _pack_dup3(wts)
        if base_variant in ("bf16_tiled", "bf16_dup3"):
            xpad = xpad.astype(ml_dtypes.bfloat16)
            wts = wts.astype(ml_dtypes.bfloat16)
        in_maps = [{"x": xpad[i], "wts": wts, "wb": wb} for i in range(NCORES)]
    trace = os.environ.get("BASS_KERNEL_TRACE", "0") == "1"
    if trace:
        _ensure_ntff_hook()
    res = run_bass_kernel_spmd(
        _cached_nc, in_maps, list(range(NCORES)), trace=trace
    )
    last_results = res
    out = np.stack(
        [np.asarray(res.results[i]["out"], np.float32) for i in range(NCORES)],
        axis=0,
    )
    return out



# revision 52
# speedup vs baseline: 1.0389x; 1.0389x over previous
"""Trainium2 Bass kernel for nn_CPADConvOffsetStage.

The reference module is:
  up_posi = grid_sample_bilinear_border(posi_map -> [B,16,GP,GP], grid = base + offset*scale)
  h       = relu(w1 @ up_posi + b1)           (1x1 conv)
  weights = (w2 @ h + b2).reshape(B,64,9,H,W) (1x1 conv -> per-pixel 3x3 kernels)
  x_adapt = w_ca @ x                          (1x1 conv)
  out     = sum_k weights[:,:,k] * unfold3x3(x_adapt)[:,:,k] + bias

In setup_inputs() posi_map is spatially constant per channel (jnp.ones).
Bilinear interpolation weights sum to exactly 1, so up_posi is spatially
constant => h, weights are spatially constant => the whole module reduces
to ONE dense 3x3 convolution with host-precomputable weights
    Wfull[o,c,k] = wk[o,k] * w_ca[o,c],   wk = (w2 @ relu(w1 @ v + b1) + b2)
plus the bias.  The kernel below runs that conv data-parallel over batch
(1 batch image per NeuronCore, 8 cores).

If posi_map is NOT per-channel spatially constant (never the case for the
shipped setup_inputs), we fall back to an exact numpy port of the
reference.
"""

import os
import numpy as np
from contextlib import ExitStack

import concourse.bass as bass
import concourse.tile as tile
from concourse import mybir
from concourse.bass_utils import run_bass_kernel_spmd

# Problem constants (hardcoded per contract)
B, C, H, W = 8, 64, 128, 128
OC = 64
KK = 3
POSI_CH, GP = 16, 16
NCORES = 8
F32 = mybir.dt.float32

HPAD, WPAD = H + 2, W + 2      # host-padded image (130 x 130)
ROWS_PER_TILE = 4              # 4 rows * 128 cols = 512 = max fp32 moving free dim
NFREE = ROWS_PER_TILE * W      # 512
RB = 16                        # output rows per SBUF block
NBLK = H // RB                 # blocks per image
SUB = RB // ROWS_PER_TILE      # psum tiles per block
NXB = 4                        # rotated input block buffers
F32R = mybir.dt.float32r       # fp32 storage, single-pass relaxed-precision matmul

_cached_nc = None
_cached_variant = None
last_results = None            # test harness introspection


def _ensure_ntff_hook():
    """Register the axon NTFF-profile hook that this image's antenv lacks.

    run_bass_kernel_spmd(trace=True) under axon needs
    antenv.axon_hooks.get_axon_ntff_profile_hook; the hook machinery
    exists in trn_agent_boot but was never registered because
    antenv.axon_hooks is missing.  Recreate the module in sys.modules.
    """
    import sys
    import types

    if "antenv.axon_hooks" in sys.modules:
        return
    try:
        from trn_agent_boot.trn_boot import _ntff_profile_via_ctypes

        hook = _ntff_profile_via_ctypes("/opt/axon/libaxon_pjrt.so")
    except Exception:
        hook = None
    mod = types.ModuleType("antenv.axon_hooks")
    mod.get_axon_ntff_profile_hook = lambda: hook
    mod.set_axon_ntff_profile_hook = lambda h: None
    sys.modules["antenv.axon_hooks"] = mod
    try:
        import antenv

        antenv.axon_hooks = mod
    except Exception:
        pass


def _build_conv_nc(variant="bf16_tiled", split_waits=True):
    """3x3 conv, 64->64 ch, on one (host-padded) [64,130,130] image.

    SPMD over 8 cores, one batch image per core.  The input arrives
    zero-padded from the host so every 3x3 tap is a plain shifted
    window read; no memsets / halo special cases on device (this also
    keeps every Matmult at <=2 semaphore waits — walrus rejects 3+ on
    the LDWEIGHTS struct).
    """
    o16 = variant.endswith("_o16")
    base_variant = variant[:-4] if o16 else variant
    tiled = base_variant in ("bf16_tiled", "bf16_dup3")
    dup = base_variant == "bf16_dup3"
    mm_dt = mybir.dt.bfloat16 if tiled else F32R
    o_dt = mybir.dt.bfloat16 if o16 else F32
    xb_parts = 128 if dup else C
    w_cols = 6 * OC if dup else 9 * OC
    nc = bass.Bass()
    x_d = nc.declare_dram_parameter("x", [C, HPAD, WPAD], mm_dt, isOutput=False)
    w_d = nc.declare_dram_parameter("wts", [xb_parts, w_cols], mm_dt, isOutput=False)
    b_d = nc.declare_dram_parameter("wb", [OC, 1], F32, isOutput=False)
    o_d = nc.declare_dram_parameter("out", [OC, H, W], o_dt, isOutput=True)

    with ExitStack() as ctx:
        tc = ctx.enter_context(tile.TileContext(nc))
        singles = ctx.enter_context(tc.tile_pool(name="singles", bufs=1))
        outs = ctx.enter_context(tc.tile_pool(name="outs", bufs=4))
        psum = ctx.enter_context(
            tc.tile_pool(name="psum", bufs=8 if dup else 4, space="PSUM")
        )
        tmps = ctx.enter_context(tc.tile_pool(name="tmps", bufs=4))

        w_sb = singles.tile([xb_parts, w_cols], mm_dt)
        nc.sync.dma_start(out=w_sb[:, :], in_=w_d[:, :])
        b_sb = singles.tile([OC, 1], F32)
        nc.sync.dma_start(out=b_sb[:, :], in_=b_d[:, :])

        # Rotated input blocks: 18 padded rows each (16 output rows + halo)
        xbs = []
        for i in range(NXB):
            xb_buf = singles.tile([xb_parts, RB + 2, WPAD], mm_dt, tag=f"xb{i}", name=f"xb{i}")
            if dup:
                # The duplicate (shifted) half never receives its last
                # column from DMA; zero it once so K=128 solo matmuls
                # (whose bottom-half weights are zero) can't hit NaN*0.
                nc.vector.memset(xb_buf[C : 2 * C, :, WPAD - 1 : WPAD], 0.0)
            xbs.append(xb_buf)

        # Tap k -> column group: A (psum partitions 0:64, tile_position (0,0))
        # gets taps 0,2,4,6,8; B (64:128, (0,64)) gets 1,3,5,7.  The two
        # column groups of the 128x128 PE array run concurrently (separate
        # XBUS streams), so 9 taps cost ~5 matmul slots instead of 9.
        for blk in range(NBLK):
            xb = xbs[blk % NXB]
            # Prefetch the input pipeline aggressively: schedule each block's
            # load + shifted-duplicate copy as if issued a block earlier, so
            # the first matmul of a block never waits on them (the profile
            # showed ~2.5us PE gaps at early block boundaries).
            with tc.high_priority(offset=60):
                nc.sync.dma_start(
                    out=xb[0:C, :, :],
                    in_=x_d[:, blk * RB : blk * RB + RB + 2, :],
                )
                if dup:
                    # partitions 64:128 = same rows shifted one column left,
                    # so a K=128 matmul contracts two horizontally-adjacent
                    # taps at once.  Derived on-chip (cross-partition copy)
                    # instead of a second DRAM read — DMA is the bottleneck.
                    nc.vector.tensor_copy(
                        xb[C : 2 * C, :, 0 : WPAD - 1],
                        xb[0:C, :, 1:WPAD],
                    )
            o_blk = outs.tile([OC, RB * W], o_dt)
            for s in range(SUB):
                r0 = s * ROWS_PER_TILE        # row offset within block
                if dup:
                    ps = psum.tile([128, NFREE], F32)
                    # 6 all-K=128 matmuls -> 3 col-tiled slots (K=128 streams
                    # ~2x faster per column than K=64 on this part, and the
                    # solo taps' bottom-half weights are zero):
                    #  A: pair(0,1) | pair(6,7) | solo k5
                    #  B: pair(3,4) | solo k2   | solo k8
                    mms = [
                        (0, 0, 0, True, False),   # pair row 0 -> A
                        (1, 1, 1, True, False),   # pair row 1 -> B
                        (2, 2, 0, False, False),  # pair row 2 -> A
                        (3, 0, 1, False, False),  # k2 (row 0, col 2) -> B
                        (4, 1, 0, False, True),   # k5 (row 1, col 2) -> A
                        (5, 2, 1, False, True),   # k8 (row 2, col 2) -> B
                    ]
                    for wi, i, col, st, sp in mms:
                        j = 0 if wi < 3 else 2
                        rhs = xb[:, r0 + i : r0 + i + ROWS_PER_TILE, j : j + W]
                        nc.tensor.matmul(
                            ps[64 * col : 64 * col + OC, :],
                            lhsT=w_sb[:, wi * OC : (wi + 1) * OC],
                            rhs=rhs,
                            start=st,
                            stop=sp,
                            tile_position=(0, 64 * col),
                            skip_group_check=True,
                        )
                elif tiled:
                    ps = psum.tile([128, NFREE], F32)
                    for k in range(9):
                        i, j = divmod(k, 3)
                        rhs = xb[:, r0 + i : r0 + i + ROWS_PER_TILE, j : j + W]
                        col = k % 2           # even taps -> A, odd -> B
                        nc.tensor.matmul(
                            ps[64 * col : 64 * col + OC, :],
                            lhsT=w_sb[:, k * OC : (k + 1) * OC],
                            rhs=rhs,
                            start=(k < 2),
                            stop=(k >= 7),
                            tile_position=(0, 64 * col),
                            skip_group_check=True,
                        )
                if tiled:
                    # Fold: B half evicted by ACT (with bias), A half added
                    # by DVE (walrus allows only one PSUM input per DVE op).
                    ps_a = ps[0:OC, 0:NFREE]
                    ps_b = ps[64:128, 0:NFREE]
                    tmpb = tmps.tile([OC, NFREE], F32)
                    nc.scalar.activation(
                        out=tmpb[:, :],
                        in_=ps_b,
                        func=mybir.ActivationFunctionType.Identity,
                        bias=b_sb[:, 0:1],
                        scale=1.0,
                    )
                    nc.vector.tensor_add(
                        o_blk[:, s * NFREE : (s + 1) * NFREE],
                        ps_a,
                        tmpb[:, :],
                    )
                else:
                    ps = psum.tile([OC, NFREE], F32)
                    for k in range(9):
                        i, j = divmod(k, 3)
                        rhs = xb[:, r0 + i : r0 + i + ROWS_PER_TILE, j : j + W]
                        nc.tensor.matmul(
                            ps[:, :],
                            lhsT=w_sb[:, k * OC : (k + 1) * OC],
                            rhs=rhs,
                            start=(k == 0),
                            stop=(k == 8),
                        )
                    nc.scalar.activation(
                        out=o_blk[:, s * NFREE : (s + 1) * NFREE],
                        in_=ps[:, :],
                        func=mybir.ActivationFunctionType.Identity,
                        bias=b_sb[:, 0:1],
                        scale=1.0,
                    )
            # Two half-block output DMAs: the first half drains while the
            # second half is still being computed (shorter kernel tail).
            HRB = RB // 2
            for h in range(2):
                nc.sync.dma_start(
                    out=o_d[:, blk * RB + h * HRB : blk * RB + (h + 1) * HRB, :],
                    in_=o_blk[:, h * HRB * W : (h + 1) * HRB * W].rearrange(
                        "p (r w) -> p r w", r=HRB
                    ),
                )
    if split_waits:
        _split_sync_waits(nc)
    return nc


def _split_sync_waits(nc, limit=1):
    """Hoist extra sync waits onto injected wait-only EventSemaphore ops.

    The neuronxcc walrus used under axon rejects compute instructions
    carrying more than one sync wait ("Too many sync wait commands", e.g.
    S3_LW / S3D3_AC structs).  Tile's sem assignment emits up to ~3.
    For every instruction with >limit waits, keep the first `limit` and
    prepend one wait-only EventSemaphore per extra wait on the same
    engine (same program position => same semantics).
    """
    import copy as _copy

    f = nc.m.functions[0]
    template = None
    for blk in f.blocks:
        for inst in blk.instructions:
            if type(inst).__name__ == "InstEventSemaphore":
                template = inst
                break
        if template is not None:
            break
    if template is None:
        return
    n_split = 0
    for blk in f.blocks:
        new_list = []
        changed = False
        for inst in blk.instructions:
            si = getattr(inst, "sync_info", None)
            op = str(getattr(inst, "opcode", ""))
            waits = list(si.on_wait) if (si and si.on_wait) else []
            if len(waits) > limit:
                for w in waits[limit:]:
                    ev = _copy.deepcopy(template)
                    ev.name = f"waitsplit_{n_split}"
                    n_split += 1
                    ev.engine = inst.engine
                    ev.sync_info = mybir.SyncInfo(on_wait=[w], on_update=[])
                    new_list.append(ev)
                inst.sync_info = mybir.SyncInfo(
                    on_wait=waits[:limit], on_update=list(si.on_update or [])
                )
                changed = True
            new_list.append(inst)
        if changed:
            blk.instructions = new_list


def _dma_out_engine(nc):
    """Engine whose queue carries output-DMA triggers (env-selectable)."""
    name = os.environ.get("BASS_OUT_DMA_ENGINE", "gpsimd")
    return getattr(nc, name)


def _build_spatial_nc(nwarm=12):
    """3x3 conv via spatially-split PE column groups (no PSUM fold).
    Measured 37.2us on trn2 (vs 49.7us for the fold-based baseline).

    One [64,130,130] host-padded bf16 image per core.  The two 64-wide
    PE column groups compute DIFFERENT output rows (A: first half of a
    round, B: second half), each accumulating all 9 taps of its tile
    into its own PSUM half — the baseline's expensive ACT+DVE fold of
    partial sums disappears; a single ACT Identity(+bias) evicts both
    halves at once.

    Taps: 3 horizontal pairs as K=128 matmuls (partitions 64:128 hold a
    one-column-left-shifted on-chip duplicate of the image), and the 3
    remaining taps as true K=64 matmuls row-tiled to PE row positions
    0/64 (the shifted duplicate can serve column-2 taps at row position
    64), so 4 K=64 quadrant matmuls stream concurrently.  Streamed
    columns per 4-row tile: 3*512 (pairs) + 1.5*512 (solos) = 2304
    vs 3072 for the zero-padded-solo variant.

    Rounds over output rows: 16 + 32 + 32 + 32 + 16 (small first round
    so real matmuls start early; small last round for a short tail).
    ~18 warmup matmuls on a zeroed scratch tile keep the PE HAM
    un-throttled while the first input block loads.  Output is bf16.
    """
    solo_mode = os.environ.get("BASS_SOLO_MODE", "rt")
    x_fp8 = os.environ.get("BASS_X_FP8", "0") == "1"
    nc = bass.Bass()
    BF16 = mybir.dt.bfloat16
    XDT = mybir.dt.float8e4 if x_fp8 else BF16
    x_d = nc.declare_dram_parameter("x", [C, HPAD, WPAD], XDT, isOutput=False)
    # one combined constant tensor (one DMA trigger): pair weights 0:192,
    # solo weights 192:384, per-channel bias col 384
    w_d = nc.declare_dram_parameter("w", [128, 6 * OC + 1], BF16, isOutput=False)
    o_d = nc.declare_dram_parameter("out", [OC, H, W], BF16, isOutput=True)

    # rounds: (start_row, n_rows) — two small rounds first so early
    # compute starts on little data while the input pipeline ramps
    rounds = [(0, 16), (16, 16), (32, 32), (64, 32), (96, 32)]

    with ExitStack() as ctx:
        tc = ctx.enter_context(tile.TileContext(nc))
        singles = ctx.enter_context(tc.tile_pool(name="singles", bufs=1))
        xbig = ctx.enter_context(tc.tile_pool(name="xbig", bufs=3))
        xsmall = ctx.enter_context(tc.tile_pool(name="xsmall", bufs=2))
        obig = ctx.enter_context(tc.tile_pool(name="obig", bufs=3))
        osmall = ctx.enter_context(tc.tile_pool(name="osmall", bufs=2))
        psum = ctx.enter_context(tc.tile_pool(name="psum", bufs=7, space="PSUM"))
        wpsum = ctx.enter_context(tc.tile_pool(name="wpsum", bufs=1, space="PSUM"))

        w_sb = singles.tile([128, 6 * OC + 1], BF16)
        scratch = singles.tile([128, 512], BF16)
        with tc.high_priority():
            nc.gpsimd.memset(scratch[:, :], 0.0)
        with tc.high_priority(offset=2000):
            # weights must beat the (also-hoisted) input blocks into the
            # DMA queues — everything downstream waits on them
            nc.sync.dma_start(out=w_sb[:, :], in_=w_d[:, :])
        wp_sb = w_sb[:, 0 : 3 * OC]
        ws_sb = w_sb[:, 3 * OC : 6 * OC]
        b_sb = w_sb[:, 6 * OC : 6 * OC + 1]
        b32_sb = singles.tile([128, 1], F32)
        nc.gpsimd.tensor_copy(b32_sb[:, :], b_sb)   # DVE wants fp32 bias

        # PE warmup: un-throttle the HAM clock gate while input DMA runs
        # (the warmup PSUM bank is never read).
        wps = wpsum.tile([128, 512], F32)
        for i in range(nwarm):
            g = 64 * (i % 2)
            nc.tensor.matmul(
                wps[g : g + OC, :],
                lhsT=scratch[:, 0:OC],
                rhs=scratch[:, :],
                start=True,
                stop=True,
                tile_position=(0, g),
                skip_group_check=True,
            )

        for R, NR in rounds:
            nrp = NR + 2                       # padded input rows incl. halo
            half = NR // 2                     # rows per column group
            nchunk = NR // 16                  # 2 tiles/group per chunk
            xpool = xbig if NR == 32 else xsmall
            opool = obig if NR == 32 else osmall
            xb = xpool.tile([128, nrp, WPAD], XDT)
            # round 0: split the load in two so the A-lane (first `half`
            # rows + halo) can start while the B half is still in flight,
            # and outrank even the weights in the DMA queues
            splits = (
                [(0, half + 2), (half + 2, nrp)] if R == 0 else [(0, nrp)]
            )
            with tc.high_priority(offset=3000 if R == 0 else 60):
                for r0_, r1_ in splits:
                    nc.sync.dma_start(
                        out=xb[0:C, r0_:r1_, :],
                        in_=x_d[:, R + r0_ : R + r1_, :],
                    )
                    # partitions 64:128 = one column left-shifted duplicate
                    # so a K=128 matmul contracts two adjacent taps.
                    nc.vector.tensor_copy(
                        xb[C : 2 * C, r0_:r1_, 0 : WPAD - 1],
                        xb[0:C, r0_:r1_, 1:WPAD],
                    )
                if solo_mode == "pad":
                    # zero-padded K=128 solos read dup col WPAD-1 (weights
                    # are zero there but NaN*0 would poison the sum)
                    nc.vector.memset(xb[C : 2 * C, :, WPAD - 1 : WPAD], 0.0)
            o_blk = opool.tile([128, half * W], BF16)

            ntile = 2 * nchunk                 # tiles per column group
            # PSUM bank t: partitions 0:64 = group-A tile t (round rows
            # 4t..4t+4), partitions 64:128 = group-B tile t (+half rows).
            pss = [
                psum.tile([128, NFREE], F32, tag="ps", name=f"ps{R}_{k}")
                for k in range(ntile)
            ]

            def pair_mm(j, t, g):
                r0 = 4 * t + (half if g else 0)
                return (
                    xb[0 : 2 * C, r0 + j : r0 + j + 4, 0:W],
                    wp_sb[:, j * OC : (j + 1) * OC],
                    (0, g),
                )

            def solo_mm(i, t, g, rp):
                """Tap 3i+2 as K=64 at PE row position rp (0 or 64);
                the shifted duplicate serves it at rp=64 (column 1)."""
                r0 = 4 * t + (half if g else 0)
                if solo_mode == "pad":
                    return (
                        xb[0 : 2 * C, r0 + i : r0 + i + 4, 2:WPAD],
                        ws_sb[:, i * OC : (i + 1) * OC],
                        (0, g),
                    )
                if rp == 0:
                    rhs = xb[0:C, r0 + i : r0 + i + 4, 2:WPAD]
                else:
                    rhs = xb[C : 2 * C, r0 + i : r0 + i + 4, 1 : WPAD - 1]
                return (
                    rhs,
                    ws_sb[rp : rp + C, i * OC : (i + 1) * OC],
                    (rp, g),
                )

            # All solos of the round first, then all pairs: the PE drains
            # on every 128-row <-> 64-row-tiled mode switch (~150ns), so
            # switch once per round, not once per chunk.  Solos alternate
            # tile parity = alternating row position AND psum bank — row
            # tiles may run concurrently only on different banks.
            rpb = 0 if solo_mode == "rt0" else 64
            seqs = []
            for g in (0, 64):
                lst = []
                for t in range(ntile):
                    rp = rpb if (t % 2) else 0
                    for i in range(3):
                        lst.append((t, solo_mm(i, t, g, rp), i == 0, False))
                for t in range(ntile):
                    for j in range(3):
                        lst.append((t, pair_mm(j, t, g), False, j == 2))
                # reorder solos: interleave tile pairs (t even, t odd) so
                # consecutive solos hit different row positions and banks
                so = []
                for ck in range(nchunk):
                    a = lst[6 * ck : 6 * ck + 3]
                    bl = lst[6 * ck + 3 : 6 * ck + 6]
                    for x, y in zip(a, bl):
                        so.extend((x, y))
                seqs.append(so + lst[6 * nchunk :])
            for mmA, mmB in zip(*seqs):
                for g, (t, m, st, sp) in ((0, mmA), (64, mmB)):
                    rhs, lhsT, tpos = m
                    nc.tensor.matmul(
                        pss[t][g : g + OC, :],
                        lhsT=lhsT,
                        rhs=rhs,
                        start=st,
                        stop=sp,
                        tile_position=(tpos[0], g),
                        skip_group_check=True,
                    )
            # evict each bank (both column-group halves + bias in one op),
            # alternating Scalar/Vector so neither engine gates the tail
            for k in range(ntile):
                dst = o_blk[:, k * NFREE : (k + 1) * NFREE]
                if k % 2 == 0:
                    nc.scalar.activation(
                        out=dst,
                        in_=pss[k][:, :],
                        func=mybir.ActivationFunctionType.Identity,
                        bias=b_sb[:, 0:1],
                        scale=1.0,
                    )
                else:
                    nc.vector.tensor_scalar_add(
                        dst, pss[k][:, :], b32_sb[:, 0:1]
                    )
            # per-chunk output DMA (starts while later chunks still compute);
            # triggers alternate gpsimd/scalar so descriptor generation
            # overlaps.  (Measured dead ends: routing late-round triggers to
            # Sync blocks the next round's input trigger behind eviction
            # waits; all-Scalar for the last round serializes ~0.5us
            # descriptor generations on the eviction engine.  Both lost more
            # than GpSimd's post-last-trigger teardown drain costs.)
            for ck in range(nchunk):
                rows = 8 if nchunk > 1 else half
                for g, roff in ((0, 0), (64, half)):
                    if R == rounds[-1][0]:
                        # final round only: Sync is safe (no later input
                        # trigger to block) and halves the descgen chain
                        eng = (nc.scalar, nc.sync)[(2 * ck + (g > 0)) % 2]
                    else:
                        eng = (
                            nc.scalar
                            if (ck + (g > 0)) % 2
                            else _dma_out_engine(nc)
                        )
                    eng.dma_start(
                        out=o_d[
                            :,
                            R + roff + rows * ck : R + roff + rows * (ck + 1),
                            :,
                        ],
                        in_=o_blk[
                            g : g + OC, ck * 2 * NFREE : (ck + 1) * 2 * NFREE
                        ].rearrange("p (r w) -> p r w", r=rows),
                    )
    _split_sync_waits(nc)
    return nc


def _pack_spatial(wts, solo_mode="rt"):
    """Split tap-major [C, 9*OC] lhsT into pair/solo stationary layouts.

    w_pairs [128, 3*OC]: K=128 blocks, taps (3j, 3j+1) stacked on the
    partition axis (base image / shifted duplicate).
    w_solo  [128, 3*OC]: solo taps (3i+2); duplicated on BOTH partition
    halves for rt mode (same weights serve PE row positions 0 and 64),
    zero bottom half for pad mode (K=128 matmuls, dup half inert).
    """
    wpair = np.zeros((128, 3 * OC), np.float32)
    wsolo = np.zeros((128, 3 * OC), np.float32)
    for j in range(3):
        wpair[0:C, j * OC:(j + 1) * OC] = wts[:, (3 * j) * OC:(3 * j + 1) * OC]
        wpair[C:2 * C, j * OC:(j + 1) * OC] = wts[:, (3 * j + 1) * OC:(3 * j + 2) * OC]
        solo = wts[:, (3 * j + 2) * OC:(3 * j + 3) * OC]
        wsolo[0:C, j * OC:(j + 1) * OC] = solo
        if solo_mode != "pad":
            wsolo[C:2 * C, j * OC:(j + 1) * OC] = solo
    return wpair, wsolo


def _host_conv_weights(posi_map, w1, b1, w2, b2, w_ca, bias):
    """Collapse the constant-posi_map weight generator on the host."""
    pm = np.asarray(posi_map, np.float64)[0]              # [16, GP, GP]
    vvec = pm.reshape(POSI_CH, -1)[:, 0]                  # per-channel constant
    h = np.maximum(np.asarray(w1, np.float64) @ vvec + np.asarray(b1, np.float64), 0.0)
    wvec = np.asarray(w2, np.float64) @ h + np.asarray(b2, np.float64)   # [576]
    wk = wvec.reshape(OC, 9)                              # [o, k]
    wca = np.asarray(w_ca, np.float64)                    # [o, c]
    wfull = wk[:, None, :] * wca[:, :, None]              # [o, c, k]
    wts = np.ascontiguousarray(
        wfull.transpose(1, 2, 0).reshape(C, 9 * OC).astype(np.float32)
    )                                                     # [c, k*OC + o]
    wb = np.ascontiguousarray(
        np.asarray(bias, np.float32).reshape(OC, 1)
    )
    return wts, wb


def _pack_dup3(wts):
    """Repack [C, 9*OC] tap-major lhsT into the dup3 layout [128, 6*OC].

    Columns 0:3*OC are K=128 pairs (taps (3p, 3p+1) stacked on the
    partition axis, matching the +1-column-shifted input duplicate);
    columns 3*OC:6*OC are the K=64 solo taps (3q+2), bottom half zero.
    """
    w3 = np.zeros((128, 6 * OC), np.float32)
    for p in range(3):
        w3[0:C, p * OC:(p + 1) * OC] = wts[:, (3 * p) * OC:(3 * p + 1) * OC]
        w3[C:2 * C, p * OC:(p + 1) * OC] = wts[:, (3 * p + 1) * OC:(3 * p + 2) * OC]
        w3[0:C, (3 + p) * OC:(4 + p) * OC] = wts[:, (3 * p + 2) * OC:(3 * p + 3) * OC]
    return w3


def _numpy_reference(x, offset, posi_map, w1, b1, w2, b2, w_ca, bias):
    """Exact numpy port of reference.py (general-input fallback)."""
    x = np.asarray(x, np.float32)
    offset = np.asarray(offset, np.float32)
    posi_map = np.asarray(posi_map, np.float32)
    w1 = np.asarray(w1, np.float32)
    b1 = np.asarray(b1, np.float32)
    w2 = np.asarray(w2, np.float32)
    b2 = np.asarray(b2, np.float32)
    w_ca = np.asarray(w_ca, np.float32)
    bias = np.asarray(bias, np.float32)

    Bq, _, Hq, Wq = x.shape
    dx = offset[:, 0] * (2.0 / max(Wq - 1, 1)) * 0.5
    dy = offset[:, 1] * (2.0 / max(Hq - 1, 1)) * 0.5
    ys = np.linspace(-1.0, 1.0, Hq, dtype=x.dtype)
    xs = np.linspace(-1.0, 1.0, Wq, dtype=x.dtype)
    gx = xs[None, None, :] + dx
    gy = ys[None, :, None] + dy
    img = np.broadcast_to(posi_map, (Bq, posi_map.shape[1], GP, GP))

    Hp = Wp = GP
    imgT = img.transpose(0, 2, 3, 1)                      # [B, Hp, Wp, C]
    ix = np.clip((gx + 1.0) * 0.5 * (Wp - 1), 0.0, Wp - 1)
    iy = np.clip((gy + 1.0) * 0.5 * (Hp - 1), 0.0, Hp - 1)
    x0 = np.floor(ix).astype(np.int32)
    y0 = np.floor(iy).astype(np.int32)
    x1 = np.minimum(x0 + 1, Wp - 1)
    y1 = np.minimum(y0 + 1, Hp - 1)
    wx = (ix - x0.astype(ix.dtype))[..., None]
    wy = (iy - y0.astype(iy.dtype))[..., None]
    bb = np.arange(Bq)[:, None, None]
    v00 = imgT[bb, y0, x0]
    v01 = imgT[bb, y0, x1]
    v10 = imgT[bb, y1, x0]
    v11 = imgT[bb, y1, x1]
    top = v00 * (1 - wx) + v01 * wx
    bot = v10 * (1 - wx) + v11 * wx
    up = (top * (1 - wy) + bot * wy).transpose(0, 3, 1, 2)  # [B, 16, H, W]

    h = np.maximum(np.einsum('oc,bchw->bohw', w1, up) + b1[None, :, None, None], 0.0)
    weights = np.einsum('oc,bchw->bohw', w2, h) + b2[None, :, None, None]
    weights = weights.reshape(Bq, OC, KK * KK, Hq, Wq)
    x_adapt = np.einsum('oc,bchw->bohw', w_ca, x)
    xp = np.pad(x_adapt, ((0, 0), (0, 0), (1, 1), (1, 1)))
    patches = np.stack(
        [xp[:, :, i:i + Hq, j:j + Wq] for i in range(KK) for j in range(KK)],
        axis=2,
    )
    out = (weights * patches).sum(axis=2) + bias
    return out.astype(np.float32)


def kernel(**inputs):
    global _cached_nc, last_results
    x = np.ascontiguousarray(np.asarray(inputs["x"], np.float32))
    posi_map = np.asarray(inputs["posi_map"], np.float32)

    per_ch = posi_map.reshape(posi_map.shape[0] * posi_map.shape[1], -1)
    if not np.all(per_ch == per_ch[:, :1]):
        # general (spatially varying posi_map) fallback: exact numpy port
        return _numpy_reference(**{k: inputs[k] for k in (
            "x", "offset", "posi_map", "w1", "b1", "w2", "b2", "w_ca", "bias")})

    wts, wb = _host_conv_weights(
        posi_map, inputs["w1"], inputs["b1"], inputs["w2"], inputs["b2"],
        inputs["w_ca"], inputs["bias"],
    )

    variant = os.environ.get("BASS_KERNEL_VARIANT", "spatial_rt")
    global _cached_variant
    if _cached_nc is None or _cached_variant != variant:
        if variant == "spatial_rt":
            _cached_nc = _build_spatial_nc(
                nwarm=int(os.environ.get("BASS_NWARM", "12"))
            )
        else:
            _cached_nc = _build_conv_nc(variant)
        _cached_variant = variant

    xpad = np.pad(x, ((0, 0), (0, 0), (1, 1), (1, 1)))
    import ml_dtypes

    if variant == "spatial_rt":
        wpair, wsolo = _pack_spatial(
            wts, os.environ.get("BASS_SOLO_MODE", "rt")
        )
        if os.environ.get("BASS_X_FP8", "0") == "1":
            xpad = xpad.astype(ml_dtypes.float8_e4m3fn)
        else:
            xpad = xpad.astype(ml_dtypes.bfloat16)
        wall = np.concatenate(
            [wpair, wsolo, np.concatenate([wb, wb], axis=0)], axis=1
        ).astype(ml_dtypes.bfloat16)
        in_maps = [{"x": xpad[i], "w": wall} for i in range(NCORES)]
    else:
        base_variant = variant[:-4] if variant.endswith("_o16") else variant
        if base_variant == "bf16_dup3":
            wts = _pack_dup3(wts)
        if base_variant in ("bf16_tiled", "bf16_dup3"):
            xpad = xpad.astype(ml_dtypes.bfloat16)
            wts = wts.astype(ml_dtypes.bfloat16)
        in_maps = [{"x": xpad[i], "wts": wts, "wb": wb} for i in range(NCORES)]
    trace = os.environ.get("BASS_KERNEL_TRACE", "0") == "1"
    if trace:
        _ensure_ntff_hook()
    res = run_bass_kernel_spmd(
        _cached_nc, in_maps, list(range(NCORES)), trace=trace
    )
    last_results = res
    out = np.stack(
        [np.asarray(res.results[i]["out"], np.float32) for i in range(NCORES)],
        axis=0,
    )
    return out

